# revision 39
# baseline (speedup 1.0000x reference)
"""DarcyFlow operator Ax = conv2x2(4ch a-weighted shifts of x) zero-padded.

Strategy (8 NeuronCores, data-parallel over image ROWS):
  - Core c owns output rows [128c .. 128c+127] of all 16 images.
  - Planes are scaled by 1/6 (folded into the replicated `a` field) so every
    stencil weight is 6*K in {+-1, +-2, +-4} - exact in bf16 AND fp8.
  - Per window (one image's 128-row slab) the operator is 16 taps over four
    product planes P_uv = a6 (x) x at the four relative (row,col) alignments:
        A0X = a6[r]*x[r]      A1X  = a6[r-1]*x[r]
        A0sX= a6[r,c-1]*x[c]  A1sX = a6[r-1,c-1]*x[c]
    A0X/A1X are one chunked DVE tensor_tensor op; A0sX a second DVE op;
    A1sX runs on the Pool (gpsimd) engine - all bf16 (DVE 2x_1p mode).
  - Tap accumulation on the Tensor engine into PSUM: channels Q2,Q3,Q4 are
    6 banded bf16 matmuls (row shifts in the stationary band, col shifts in
    the moving-slice offset).  Channel Q1 (plane A1X) is converted to fp8-e4m3
    by the Scalar engine and its two col-shift passes merge into ONE
    fp8 DoubleRow matmul (2 k-tiles at 0.5 cyc/row): the two k-tiles are the
    fp8 plane and a one-column-shifted copy of it (SBUF->SBUF DMA) so the
    k-tile byte stride stays even (odd strides crash the PE).
    One fp8 plane keeps the extra quantization error ~1e-2 rel (<2e-2 gate).
  - Tile-0's (Q3,dj0) pass is further offloaded from the PE to DVE tap ops
    on the drained stage tile (tensor_scalar 4x mode for the x4 weight; the
    row-shifted operand comes from a partition-shifting SBUF->SBUF DMA since
    engine APs must start at partition 0).
  - ScalarE drains PSUM -> SBUF bf16; borders: output DRAM starts zeroed,
    stores skip border cols, host drops the garbage global top/bottom row.
  - Per image one 128-row window produces 126 output rows; the remaining
    2 rows/image come from one packed tail window (16 img x 4 rows) that runs
    FIRST (absorbing the DMA lead-in); main images run as X-load/store pairs
    to halve the serial HWDGE desc-gen count, with the last two windows
    single (split final stores shorten the end drain chain).
  - The `a`-derived field is sent once per core as [z | A0 | A1] (the
    col-shifted A0s/A1s variants are just offset reads of the same chunks).

  Engine balance per core (TimelineSim): PE 46.7us, DVE 42.9, Act 39.8,
  Pool 38.8, DMA 38.5, HWDGE 36.9 over a 56.9us span (baseline: 70.2us).
"""

import numpy as np
import ml_dtypes

BF16 = ml_dtypes.bfloat16
F8NP = ml_dtypes.float8_e4m3

B = 16
N = 1024
NCORES = 8
SLAB = N // NCORES  # 128
WX = N + 2          # padded x width (zero col both sides) = 1026
WQ = N + 1          # valid product width = 1025
WQE = 1026          # product chunk width
COLT = 512          # psum bank column tile
AW = 1 + 2 * WQE    # a-field tile width: [z | A0 | A1]

USE_FP8 = True      # Q1 channel via fp8 DoubleRow (False -> all-bf16 8 passes)

_K = np.array(
    [
        [[-1 / 6, 2 / 3], [-1 / 3, -1 / 6]],  # K1 (ch Q1, plane A1X)
        [[2 / 3, -1 / 6], [-1 / 6, -1 / 3]],  # K2 (Q2, plane A1sX)
        [[-1 / 6, -1 / 3], [2 / 3, -1 / 6]],  # K3 (Q3, plane A0sX)
        [[-1 / 3, -1 / 6], [-1 / 6, 2 / 3]],  # K4 (Q4, plane A0X)
    ],
    dtype=np.float32,
)
_K6 = _K * 6  # entries in {+-1, +-2, +-4}

# bf16 pass order: (channel, dj) - Q2 (the Pool-produced plane) runs LAST in
# each group so the slower Pool product is off the lead-in critical path
BF_PASSES = [(ch, dj) for dj in (0, 1) for ch in (2, 3, 1)]
ALL_PASSES = [(ch, dj) for dj in (0, 1) for ch in (2, 3, 0, 1)]


def _build_weights():
    """Host-built stationary matrices (main + tail), 6K-scaled."""
    passes = BF_PASSES if USE_FP8 else ALL_PASSES
    npass = len(passes)
    wm = np.zeros((npass, SLAB, SLAB), dtype=np.float32)
    wt = np.zeros((npass, 64, 32), dtype=np.float32)
    for p, (ch, dj) in enumerate(passes):
        off = 0 if ch < 2 else -1  # Q1/Q2 band k-m in {0,1}; Q3/Q4 in {-1,0}
        for m in range(1, SLAB - 1):
            for di in range(2):
                wm[p, m + off + di, m] = _K6[ch, di, dj]
        for b in range(16):
            for u in range(2):
                for di in range(2):
                    t = u + di + (1 if ch < 2 else 0)
                    wt[p, 4 * b + t, 2 * b + u] = _K6[ch, di, dj]
    # DoubleRow stationaries for Q1 (off 0): k-tile t = dj band
    wdr = np.zeros((SLAB, 2, SLAB), dtype=np.float32)
    wdrt = np.zeros((64, 2, 32), dtype=np.float32)
    for t in range(2):
        for m in range(1, SLAB - 1):
            for di in range(2):
                wdr[m + di, t, m] = _K6[0, di, t]
        for b in range(16):
            for u in range(2):
                for di in range(2):
                    tt = u + di + 1
                    wdrt[4 * b + tt, t, 2 * b + u] = _K6[0, di, t]
    return (
        np.ascontiguousarray(wm.transpose(1, 0, 2).reshape(SLAB, npass * SLAB)).astype(BF16),
        np.ascontiguousarray(wt.transpose(1, 0, 2).reshape(64, npass * 32)).astype(BF16),
        np.ascontiguousarray(wdr.reshape(SLAB, 2 * SLAB)).astype(F8NP),
        np.ascontiguousarray(wdrt.reshape(64, 2 * 32)).astype(F8NP),
    )


def _shard_inputs(x, a):
    """Per-core padded bf16 input arrays. x: [B,1,N,N], a: [1,1,N-1,N-1]."""
    x = np.asarray(x, dtype=np.float32).reshape(B, N, N)
    a6 = np.asarray(a, dtype=np.float32).reshape(N - 1, N - 1) / 6.0

    # zero-padded a6 lookup: row r+1 = a6[r], cols 1..N-1
    apad = np.zeros((N + 2, WQ), dtype=np.float32)
    apad[1:N, 1:N] = a6

    def afield(rows0, rows1):
        """[len(rows0), AW] = [z | A0 | A1]; A0[k]=a6row(rows0[k]),
        A1[k]=a6row(rows1[k]); each chunk has a trailing zero col."""
        m = np.zeros((len(rows0), AW), dtype=np.float32)
        for k, (r0_, r1_) in enumerate(zip(rows0, rows1)):
            m[k, 1 : 1 + WQ] = apad[r0_ + 1]
            m[k, 1 + WQE : 1 + WQE + WQ] = apad[r1_ + 1]
        return m.astype(BF16)

    wm, wt, wdr, wdrt = _build_weights()
    xpad = np.zeros((N + 2, WX), dtype=np.float32)
    xpadB = np.zeros((B, N + 2, WX), dtype=np.float32)
    xpadB[:, 1 : N + 1, 1 : N + 1] = x
    shards = []
    for c in range(NCORES):
        r0 = c * SLAB
        # window rows r0-1+k, k=0..127 (xpadB row index r0+k)
        xc = np.ascontiguousarray(xpadB[:, r0 : r0 + SLAB, :]).astype(BF16)
        # tail: partition 4b+t <- x row r0+124+t (t=0..3)
        xt = np.ascontiguousarray(
            xpadB[:, r0 + SLAB - 2 : r0 + SLAB + 2, :].reshape(64, WX)
        ).astype(BF16)
        am = afield([r0 - 1 + k for k in range(SLAB)],
                    [r0 - 2 + k for k in range(SLAB)])
        at = afield([r0 + 125 + t for _ in range(16) for t in range(4)],
                    [r0 + 124 + t for _ in range(16) for t in range(4)])
        shards.append(
            {
                "xc": xc,
                "xt": xt,
                "am": np.ascontiguousarray(am),
                "at": np.ascontiguousarray(at),
                "wm": wm,
                "wt": wt,
                **({"wdr": wdr, "wdrt": wdrt} if USE_FP8 else {}),
            }
        )
    return shards


_CACHE = {}


def _build_module(iters=1, variant="full"):
    """Build + compile the (identical-program) per-core Bass module."""
    key = ("nc", iters, variant, USE_FP8)
    if key in _CACHE:
        return _CACHE[key]

    import concourse.bacc as bacc
    import concourse.tile as tile
    from concourse import mybir

    bf16 = mybir.dt.bfloat16
    f8 = mybir.dt.float8e4
    npass = len(BF_PASSES) if USE_FP8 else len(ALL_PASSES)

    nc = bacc.Bacc("TRN2", target_bir_lowering=False, debug=False,
                   num_devices=NCORES)

    xc_d = nc.dram_tensor("xc", [B, SLAB, WX], bf16, kind="ExternalInput").ap()
    xt_d = nc.dram_tensor("xt", [64, WX], bf16, kind="ExternalInput").ap()
    am_d = nc.dram_tensor("am", [SLAB, AW], bf16, kind="ExternalInput").ap()
    at_d = nc.dram_tensor("at", [64, AW], bf16, kind="ExternalInput").ap()
    wm_d = nc.dram_tensor("wm", [SLAB, npass * SLAB], bf16, kind="ExternalInput").ap()
    wt_d = nc.dram_tensor("wt", [64, npass * 32], bf16, kind="ExternalInput").ap()
    if USE_FP8:
        wdr_d = nc.dram_tensor("wdr", [SLAB, 2 * SLAB], f8, kind="ExternalInput").ap()
        wdrt_d = nc.dram_tensor("wdrt", [64, 2 * 32], f8, kind="ExternalInput").ap()
    out_d = nc.dram_tensor("out", [B, SLAB, N], bf16, kind="ExternalOutput").ap()
    outt_d = nc.dram_tensor("outt", [32, N], bf16, kind="ExternalOutput").ap()

    with tile.TileContext(nc) as tc:
        with (
            tc.tile_pool(name="const", bufs=1) as const,
            tc.tile_pool(name="xin", bufs=4) as xin,
            tc.tile_pool(name="prod", bufs=4) as prod,
            tc.tile_pool(name="stage", bufs=4) as stage,
            tc.tile_pool(name="psum", bufs=6, space="PSUM") as psum,
        ):
            # a-field loads on the gpsimd queue: separate desc-gen stream
            # so X loads on the sync queue overlap them
            At = const.tile([64, AW], bf16)
            nc.gpsimd.dma_start(At[:], at_d[:])
            Am = const.tile([SLAB, AW], bf16)
            nc.gpsimd.dma_start(Am[:], am_d[:])
            Wt = const.tile([64, npass * 32], bf16)
            nc.scalar.dma_start(Wt[:], wt_d[:])
            if USE_FP8:
                Wdrt = const.tile([64, 2 * 32], f8)
                nc.scalar.dma_start(Wdrt[:], wdrt_d[:])
            Wm = const.tile([SLAB, npass * SLAB], bf16)
            nc.scalar.dma_start(Wm[:], wm_d[:])
            if USE_FP8:
                Wdr = const.tile([SLAB, 2 * SLAB], f8)
                nc.scalar.dma_start(Wdr[:], wdr_d[:])

            # PE warmup: dummy matmuls bridge the DMA-bound startup window so
            # the p-state ramp is paid before the real matmul stream begins.
            if variant == "full":
                scratch = const.tile([SLAB, 64], bf16)
                nc.vector.memset(scratch[:, 0:2], 0.0)
                rhs_w = scratch[:, 0:2][:, None, :].broadcast_to([SLAB, 256, 2])
                for _ in range(8):
                    wps = psum.tile([32, COLT], mybir.dt.float32,
                                    name="ps_64", tag="ps_64", bufs=2)
                    nc.tensor.matmul(wps[:], scratch[:, 0:32], rhs_w,
                                     start=True, stop=True)

            def drpair_rhs(q8, base):
                """[K, 2, COLT] moving AP over the fp8 chunk pair (stride WQE)."""
                sl = q8[:, base : base + COLT]
                return type(sl)(sl.tensor, sl.offset,
                                [list(sl.ap[0]), [WQE, 2], [1, COLT]])

            def window(X, Af, P, M, wtile, wstride, wdrtile, st, ps_bufs,
                       store=None, defer_dr=False):
                """One banded-stencil window.
                X: [P, WX] input, Af: [P, AW]=[z|A0|A1], M: out partitions."""
                if variant == "dma":
                    return
                # products: q41 = [A0X | A1X] (DVE), qs0 = A0sX (DVE),
                # qs1 = A1sX (Pool)
                q41 = prod.tile([P, 2 * WQE], bf16, name=f"q41_{P}", tag=f"q41_{P}")
                nc.vector.tensor_mul(
                    q41[:].rearrange("p (c w) -> p c w", c=2),
                    Af[:, 1 : 1 + 2 * WQE].rearrange("p (c w) -> p c w", c=2),
                    X[:, 0:WQE][:, None, :].broadcast_to([P, 2, WQE]),
                )
                qs0 = prod.tile([P, WQE], bf16, name=f"qs0_{P}", tag=f"qs0_{P}")
                nc.vector.tensor_mul(qs0[:], Af[:, 0:WQE], X[:, 0:WQE])
                qs1 = prod.tile([P, WQE], bf16, name=f"qs1_{P}", tag=f"qs1_{P}")
                nc.gpsimd.tensor_mul(qs1[:], Af[:, WQE : 2 * WQE], X[:, 0:WQE])
                if USE_FP8:
                    q8 = prod.tile([P, 2 * WQE], f8, name=f"q8_{P}", tag=f"q8_{P}")
                    nc.scalar.copy(q8[:, 0:WQE], q41[:, WQE : 2 * WQE])
                    # chunk1[c] = chunk0[c+1]: even k-tile byte stride for DR
                    nc.sync.dma_start(q8[:, WQE : WQE + WQ], q8[:, 1 : 1 + WQ])

                # moving slice base per channel and col tile
                def moving(ch, t, dj):
                    b0 = t * COLT + dj
                    if ch == 3:  # Q4: A0X chunk of q41
                        return q41[:, b0 : b0 + COLT]
                    if ch == 2:  # Q3: A0sX
                        return qs0[:, b0 + 1 : b0 + 1 + COLT]
                    if ch == 1:  # Q2: A1sX
                        return qs1[:, b0 + 1 : b0 + 1 + COLT]
                    return q41[:, WQE + b0 : WQE + b0 + COLT]  # Q1 (bf16 mode)

                passes = BF_PASSES if USE_FP8 else ALL_PASSES
                # tile-0's (Q3, dj0) pass runs as DVE taps after the drain
                dve_taps = USE_FP8 and P == SLAB

                # engine APs must start at partition 0, so the row-shifted
                # operand comes from a small partition-shifting SBUF DMA
                if dve_taps:
                    qs0d = prod.tile([P, COLT], bf16, name="qs0d", tag="qs0d")
                    nc.sync.dma_start(qs0d[1:P, 0 : COLT - 1],
                                      qs0[0 : P - 1, 2 : COLT + 1])

                def emit_taps(t):
                    if not (dve_taps and t == 0):
                        return
                    # (Q3,dj0) taps: st[m,c] += -qs0[m-1,c+1] + 4*qs0[m,c+1]
                    # qs0d[m, c-1] = qs0[m-1, c+1] for c in [1, COLT)
                    c0, c1 = 1, COLT
                    w_ = c1 - c0
                    tmp = prod.tile([P, COLT], bf16, name="tap", tag="tap")
                    nc.vector.tensor_scalar_mul(
                        tmp[:, 0:w_], qs0[:, c0 + 1 : c1 + 1], 4.0)
                    nc.vector.tensor_sub(
                        st[:, c0:c1], st[:, c0:c1], qs0d[:, 0:w_])
                    nc.vector.tensor_add(
                        st[:, c0:c1], st[:, c0:c1], tmp[:, 0:w_])

                def emit_dr(t, ps):
                    nc.tensor.matmul(
                        ps[:],
                        wdrtile[:].rearrange("p (k m) -> p k m", k=2),
                        drpair_rhs(q8, t * COLT),
                        start=False, stop=True,
                        perf_mode=mybir.MatmulPerfMode.DoubleRow,
                    )
                    nc.scalar.copy(st[:, t * COLT : (t + 1) * COLT], ps[:])
                    emit_taps(t)
                    if store is not None:
                        store(t, st)

                pst = []
                for t in range(2):
                    ps = psum.tile([M, COLT], mybir.dt.float32,
                                   name=f"ps_{P}", tag=f"ps_{P}", bufs=ps_bufs)
                    pst.append(ps)
                    first = True
                    for p, (ch, dj) in enumerate(passes):
                        if dve_taps and t == 0 and (ch, dj) == (2, 0):
                            continue
                        nc.tensor.matmul(
                            ps[:],
                            wtile[:, p * wstride : (p + 1) * wstride],
                            moving(ch, t, dj),
                            start=first,
                            stop=(p == len(passes) - 1) and not USE_FP8,
                        )
                        first = False
                    if USE_FP8 and not defer_dr:
                        emit_dr(t, ps)
                    elif not USE_FP8:
                        nc.scalar.copy(st[:, t * COLT : (t + 1) * COLT], ps[:])
                        if store is not None:
                            store(t, st)
                if USE_FP8 and defer_dr:
                    # all bf16 passes first, DR passes last: gives the
                    # convert+shift chain time to land during the lead-in
                    for t in range(2):
                        emit_dr(t, pst[t])

            def body():
                def tail_window():
                    Xt = xin.tile([64, WX], bf16, name="xtw", tag="xtw")
                    nc.sync.dma_start(Xt[:], xt_d[:, :])
                    stt = stage.tile([32, N], bf16, name="stt", tag="stt")

                    def store_t(t, stt):
                        c0, c1 = (1, COLT) if t == 0 else (COLT, N - 1)
                        nc.sync.dma_start(outt_d[:, c0:c1], stt[:, c0:c1])

                    window(Xt, At, 64, 32, Wt, 32,
                           Wdrt if USE_FP8 else None, stt, 2, store_t,
                           defer_dr=True)

                def main_window(X1, st1, b, split_store):
                    def store_1(t, st1, b=b):
                        if split_store:
                            c0, c1 = (1, COLT) if t == 0 else (COLT, N - 1)
                            nc.sync.dma_start(out_d[b, 0 : SLAB - 2, c0:c1],
                                              st1[1 : SLAB - 1, c0:c1])
                        elif t == 1:
                            nc.sync.dma_start(out_d[b, 0 : SLAB - 2, 1 : N - 1],
                                              st1[1 : SLAB - 1, 1 : N - 1])

                    window(X1, Am, SLAB, SLAB, Wm, SLAB,
                           Wdr if USE_FP8 else None, st1, 6, store_1)

                # tail first: its passes absorb the lead-in while the first
                # main products resolve
                tail_window()

                # image pairs (0,1) .. (12,13) share X-load and store DMAs
                for bp in range(7):
                    b0 = 2 * bp
                    X2 = xin.tile([SLAB, 2, WX], bf16, name="xw", tag="xw")
                    nc.sync.dma_start(
                        X2[:], xc_d[b0 : b0 + 2, :, :].transpose([1, 0, 2]))
                    st2 = stage.tile([SLAB, 2, N], bf16, name="stm", tag="stm")

                    def store_m(t, st2, i, b0=b0):
                        if t == 1 and i == 1:
                            nc.sync.dma_start(
                                out_d[b0 : b0 + 2, 0 : SLAB - 2,
                                      1 : N - 1].transpose([1, 0, 2]),
                                st2[1 : SLAB - 1, :, 1 : N - 1])

                    for i in range(2):
                        window(X2[:, i, :], Am, SLAB, SLAB, Wm, SLAB,
                               Wdr if USE_FP8 else None, st2[:, i, :], 6,
                               lambda t, st, i=i: store_m(t, st2, i))

                # last two windows single, the final one with split stores
                for b, split in ((B - 2, False), (B - 1, True)):
                    X1 = xin.tile([SLAB, WX], bf16, name="xw0", tag="xw0")
                    nc.sync.dma_start(X1[:], xc_d[b, :, :])
                    st1 = stage.tile([SLAB, N], bf16, name="st1", tag="st1")
                    main_window(X1, st1, b, split)

            if iters == 1:
                body()
            else:
                with tc.For_i(0, iters, 1):
                    body()

    nc.compile()
    _CACHE[key] = nc
    return nc


def run(inputs, trace=False, trace_kwargs=None, iters=1, variant="full"):
    """Run the sharded kernel; returns (full_output, BassKernelResults)."""
    from concourse.bass_utils import run_bass_kernel_spmd

    nc = _build_module(iters, variant)
    in_maps = _shard_inputs(inputs["x"], inputs["a"])
    res = run_bass_kernel_spmd(
        nc,
        in_maps,
        core_ids=list(range(NCORES)),
        trace=trace,
        **(trace_kwargs or {}),
    )
    full = np.zeros((B, 1, N, N), dtype=np.float32)
    for c in range(NCORES):
        oc = np.array(res.results[c]["out"]).astype(np.float32)  # [B, SLAB, N]
        oc[:, SLAB - 2 : SLAB, :] = (
            np.array(res.results[c]["outt"]).astype(np.float32).reshape(B, 2, N)
        )
        r0 = c * SLAB
        lo = 1 if c == 0 else 0            # drop garbage global row 0
        hi = SLAB - 1 if c == NCORES - 1 else SLAB  # drop garbage row N-1
        full[:, 0, r0 + lo : r0 + hi, 1 : N - 1] = oc[:, lo:hi, 1 : N - 1]
    return full, res


def kernel(**inputs) -> np.ndarray:
    out, _ = run(inputs, trace=False)
    return out
